# revision 1
# baseline (speedup 1.0000x reference)
"""Trainium2 Bass kernel for nn_BitfieldLinear (vq_codebook).

Reference computation:
    idx   = codes & 0xFF            (basis row, 256 entries)
    r_q   = (codes >> 8) & 0xFFF
    sign  = bit20 ? -1 : +1
    scale = sign * tanh(r_q / 4095)
    W     = scale[:, None] * basis[idx]        # [8192, 4096]
    y     = x @ W.T                            # [128, 8192]

Key factorization (never materialize the 128MB W):
    Z = x @ basis.T                            # [128, 256]  tiny matmul
    y[b, j] = scale[j] * Z[b, idx[j]]          # column gather + scale

The gather+scale is itself a matmul with a scaled one-hot matrix:
    G[k, j] = scale[j] * (idx[j] == k)         # [256, 1024] per core
    y_core  = Z @ G                            # [128, 1024]
Each one-hot column has a single nonzero, so the matmul computes
scale[j] * Z[b, idx[j]] directly (one product per output).

Sharding: out_features column-parallel across 8 cores (1024 codes per
core); x and basis replicated.  Per core:
    1. stream x^T / basis^T K-tiled as fp16 (halves the memory-roofline
       traffic; ~2^-11 rel err), host pre-laid-out as per-chunk
       contiguous DRAM tensors across three DMA rings; accumulate
       Z [128, 256] in PSUM over 32 fp16 matmuls
    2. decode codes on-chip (DVE bitops + ACT tanh); build G^T tiles
       with one tensor_scalar each ((iota == idx) * scale), PE-transpose
       into G (fp32r) — hidden under the input stream
    3. PE-transpose Z, y = Z^T.T @ G via 4 fp32r matmuls, store fp16
Host reassembles y by concatenating per-core outputs (pure layout).
Overall rel err ~3e-4 (fp16 inputs dominate), vs typical 2e-2 tolerance.
"""

import sys

for _p in ("/opt/trn_rl_repo", "/opt/pypackages"):
    if _p not in sys.path:
        sys.path.insert(0, _p)

import numpy as np

import concourse.bacc as bacc
import concourse.mybir as mybir
import concourse.tile as tile
from concourse.alu_op_type import AluOpType
from concourse.bass_utils import run_bass_kernel_spmd

N_CORES = 8
BATCH = 128
IN_F = 4096
OUT_F = 8192
BASIS = 256
OPC = OUT_F // N_CORES      # 1024 output columns per core
NK = IN_F // 128            # 32 K-tiles
NT = OPC // 128             # 8 code-tiles per core
R_LEVELS = 4095.0

F32 = mybir.dt.float32
F32R = mybir.dt.float32r
BF16 = mybir.dt.bfloat16
FP16 = mybir.dt.float16
I32 = mybir.dt.int32

# K-tiles per input DMA chunk: few big chunks for ring efficiency, small
# final chunk so the PE tail after the last chunk stays small
DMA_CHUNKS = [16, 8, 6, 2]
assert sum(DMA_CHUNKS) == NK

# G^T tiles built after each chunk's matmuls (fills PE DMA-wait gaps)
G_SCHED = {0: [0, 1, 2], 1: [3, 4, 5], 2: [6, 7]}

B_CHUNKS = [(0, 8), (8, 16), (16, 26), (26, 32)]
B_ENGINES = ["sync", "sync", "gpsimd", "gpsimd"]


def build_nc():
    nc = bacc.Bacc(
        "TRN2",
        target_bir_lowering=False,
        debug=False,
        num_devices=N_CORES,
    )

    # fp16 inputs: halves the input traffic (the memory roofline) at
    # ~2^-11 relative error; fp16 range is ample for N(0,1) x and 0.02*N
    # basis, and bf16-class PE rate applies.  One DRAM tensor per DMA
    # chunk so every transfer is fully contiguous in HBM.
    x16_ds = [
        nc.dram_tensor(f"x16c{i}", [128, ch * 128], FP16, kind="ExternalInput")
        for i, ch in enumerate(DMA_CHUNKS)
    ]
    b16_ds = [
        nc.dram_tensor(f"b16c{i}", [128, (be - bs) * 256], FP16,
                       kind="ExternalInput")
        for i, (bs, be) in enumerate(B_CHUNKS)
    ]
    c128_d = nc.dram_tensor("c128", [128, NT], I32, kind="ExternalInput")
    iota_d = nc.dram_tensor("iota", [128, BASIS], F32, kind="ExternalInput")
    ident_d = nc.dram_tensor("ident", [128, 128], F32, kind="ExternalInput")
    out_d = nc.dram_tensor("out", [128, OPC], FP16, kind="ExternalOutput")

    with tile.TileContext(nc) as tc:
        with (
            tc.tile_pool(name="pool", bufs=1) as pool,
            tc.tile_pool(name="zps", bufs=1, space="PSUM") as zps,
            tc.tile_pool(name="tps", bufs=2, space="PSUM") as tps,
            tc.tile_pool(name="yps", bufs=1, space="PSUM") as yps,
        ):
            # ---- small inputs (decode + constants) on the SWDGE ring so
            # the two HWDGE rings start streaming x/basis immediately
            c128 = pool.tile([128, NT], I32)
            nc.gpsimd.dma_start(out=c128[:], in_=c128_d[:])
            iota_bc = pool.tile([128, BASIS], F32)
            nc.gpsimd.dma_start(out=iota_bc[:], in_=iota_d[:])
            ident = pool.tile([128, 128], F32)
            nc.gpsimd.dma_start(out=ident[:], in_=ident_d[:])

            # ---- decode codes -> idx (f32), scale (f32), both [128, NT]
            # (bitVec TSP ops cannot cast dtypes: mask in i32, then cast
            # via fp-ALU mult).  Emitted inside the stream loop (after
            # chunk 0) so the ACT table load for tanh does not delay the
            # scalar ring's first DMA issue.
            idx_f = pool.tile([128, NT], F32)
            scl = pool.tile([128, NT], F32)

            def emit_decode():
                idx_i = pool.tile([128, NT], I32, name="idx_i")
                nc.vector.tensor_scalar(
                    out=idx_i[:], in0=c128[:],
                    scalar1=255, scalar2=None, op0=AluOpType.bitwise_and,
                )
                nc.vector.tensor_scalar_mul(
                    out=idx_f[:], in0=idx_i[:], scalar1=1.0
                )
                rq_i = pool.tile([128, NT], I32, name="rq_i")
                nc.vector.tensor_scalar(
                    out=rq_i[:], in0=c128[:],
                    scalar1=8, scalar2=4095,
                    op0=AluOpType.logical_shift_right,
                    op1=AluOpType.bitwise_and,
                )
                rq = pool.tile([128, NT], F32, name="rq")
                nc.vector.tensor_scalar_mul(
                    out=rq[:], in0=rq_i[:], scalar1=1.0 / R_LEVELS
                )
                th = pool.tile([128, NT], F32, name="th")
                nc.scalar.activation(
                    out=th[:], in_=rq[:],
                    func=mybir.ActivationFunctionType.Tanh,
                )
                sg_i = pool.tile([128, NT], I32, name="sg_i")
                nc.vector.tensor_scalar(
                    out=sg_i[:], in0=c128[:],
                    scalar1=20, scalar2=1,
                    op0=AluOpType.logical_shift_right,
                    op1=AluOpType.bitwise_and,
                )
                sgn = pool.tile([128, NT], F32, name="sgn")
                nc.vector.tensor_scalar(
                    out=sgn[:], in0=sg_i[:],
                    scalar1=-2.0, scalar2=1.0,
                    op0=AluOpType.mult, op1=AluOpType.add,
                )
                nc.vector.tensor_tensor(
                    out=scl[:], in0=th[:], in1=sgn[:], op=AluOpType.mult,
                )

            # ---- G^T tiles: gt[t][p, k] = scale[t*128+p] * (idx[t*128+p]==k)
            # one dual-op tensor_scalar per tile, then PE-transpose into G
            # G_sb[h][k', t*128+j'] with k = h*128+k'.  Emitted interleaved
            # with the stream chunks so the transposes fill PE DMA-wait gaps.
            g_sb = [pool.tile([128, OPC], F32R, tag=f"g{h}", name=f"g_sb{h}") for h in range(2)]

            def emit_g_tile(t):
                gt = pool.tile([128, BASIS], F32, tag="gt", name=f"gt{t}")
                nc.vector.tensor_scalar(
                    out=gt[:], in0=iota_bc[:],
                    scalar1=idx_f[:, t:t + 1], scalar2=scl[:, t:t + 1],
                    op0=AluOpType.is_equal, op1=AluOpType.mult,
                )
                for h in range(2):
                    tp = tps.tile([128, 128], F32, tag="tp", name=f"tp{t}_{h}")
                    nc.tensor.transpose(
                        out=tp[:], in_=gt[:, h * 128:(h + 1) * 128],
                        identity=ident[:],
                    )
                    nc.vector.tensor_copy(
                        out=g_sb[h][:, t * 128:(t + 1) * 128], in_=tp[:]
                    )

            # ---- stream x^T / basis^T (fp16) across THREE DMA rings
            # (sync + gpsimd for basis halves, scalar for x), accumulate
            # Z [128b, 256o] in PSUM (exact fp16 products into fp32 accum)
            x16_sb = pool.tile([128, IN_F], FP16)
            b16_sb = pool.tile([128, 2 * IN_F], FP16)
            z_ps = zps.tile([128, BASIS], F32, tag="z")

            for bi, (bg, bge) in enumerate(B_CHUNKS):
                eng = nc.sync if B_ENGINES[bi] == "sync" else nc.gpsimd
                eng.dma_start(
                    out=b16_sb[:, bg * 256:bge * 256],
                    in_=b16_ds[bi][:],
                )
            g = 0
            for ci, ch in enumerate(DMA_CHUNKS):
                ge = g + ch
                nc.scalar.dma_start(
                    out=x16_sb[:, g * 128:ge * 128],
                    in_=x16_ds[ci][:],
                )
                for n in range(g, ge):
                    nc.tensor.matmul(
                        z_ps[:],
                        lhsT=x16_sb[:, n * 128:(n + 1) * 128],
                        rhs=b16_sb[:, n * 256:(n + 1) * 256],
                        start=(n == 0), stop=(n == NK - 1),
                    )
                if ci == 0:
                    emit_decode()
                for t in G_SCHED.get(ci, []):
                    emit_g_tile(t)
                g = ge

            # Z -> SBUF, PE-transpose into Z^T chunks for the y matmul
            z_sb = pool.tile([128, BASIS], F32)
            nc.vector.tensor_copy(out=z_sb[:], in_=z_ps[:])
            zt = [pool.tile([128, 128], F32R, tag=f"zt{h}", name=f"zt{h}") for h in range(2)]
            for h in range(2):
                ztp = tps.tile([128, 128], F32, tag="tp", name=f"ztp{h}")
                nc.tensor.transpose(
                    out=ztp[:], in_=z_sb[:, h * 128:(h + 1) * 128],
                    identity=ident[:],
                )
                if h == 0:
                    nc.vector.tensor_copy(out=zt[h][:], in_=ztp[:])
                else:
                    nc.scalar.copy(out=zt[h][:], in_=ztp[:])

            # ---- y = Z^T.T @ G, two N-chunks of 512 (fp32r: each one-hot
            # column is a single product, so precision loss is negligible),
            # store each as soon as its PSUM copy lands
            for nch in range(2):
                y_ps = yps.tile([128, 512], F32, tag=f"y{nch}", name=f"y_ps{nch}")
                for h in range(2):
                    nc.tensor.matmul(
                        y_ps[:],
                        lhsT=zt[h][:],
                        rhs=g_sb[h][:, nch * 512:(nch + 1) * 512],
                        start=(h == 0), stop=(h == 1),
                    )
                y_sb = pool.tile([128, 512], FP16, tag=f"ysb{nch}", name=f"y_sb{nch}")
                if nch == 0:
                    nc.vector.tensor_copy(out=y_sb[:], in_=y_ps[:])
                else:
                    nc.scalar.copy(out=y_sb[:], in_=y_ps[:])
                nc.sync.dma_start(
                    out=out_d[:, nch * 512:(nch + 1) * 512], in_=y_sb[:]
                )

    nc.compile()
    return nc


_NC = None


def _get_nc():
    global _NC
    if _NC is None:
        _NC = build_nc()
    return _NC


def make_in_maps(x, codes, basis):
    import ml_dtypes

    bf16 = ml_dtypes.bfloat16
    x = np.ascontiguousarray(x, dtype=np.float32)
    basis = np.ascontiguousarray(basis, dtype=np.float32)
    codes = np.ascontiguousarray(codes, dtype=np.int32)

    # xt[p, n*128 + m] = x[m, n*128 + p]
    xt = np.ascontiguousarray(
        x.reshape(BATCH, NK, 128).transpose(2, 1, 0).reshape(128, IN_F)
    )
    # bt[p, n*256 + o] = basis[o, n*128 + p]
    bt = np.ascontiguousarray(
        basis.reshape(BASIS, NK, 128).transpose(2, 1, 0).reshape(128, 2 * IN_F)
    )
    x16 = xt.astype(np.float16)
    b16 = bt.astype(np.float16)
    xcs, g = {}, 0
    for i, ch in enumerate(DMA_CHUNKS):
        xcs[f"x16c{i}"] = np.ascontiguousarray(x16[:, g * 128:(g + ch) * 128])
        g += ch
    bcs = {}
    for i, (bs, be) in enumerate(B_CHUNKS):
        bcs[f"b16c{i}"] = np.ascontiguousarray(b16[:, bs * 256:be * 256])

    iota = np.ascontiguousarray(
        np.tile(np.arange(BASIS, dtype=np.float32), (128, 1))
    )
    ident = np.eye(128, dtype=np.float32)

    in_maps = []
    for c in range(N_CORES):
        sh = codes[c * OPC:(c + 1) * OPC]
        # wrap-128 layout: c128[p, t] = codes[t*128 + p]
        c128 = np.ascontiguousarray(sh.reshape(NT, 128).T)
        in_maps.append(
            {
                **xcs, **bcs,
                "c128": c128, "iota": iota, "ident": ident,
            }
        )
    return in_maps


def assemble_output(results):
    return np.concatenate(
        [results[c]["out"].astype(np.float32) for c in range(N_CORES)], axis=1
    )


def kernel(x, codes, basis):
    nc = _get_nc()
    in_maps = make_in_maps(x, codes, basis)
    res = run_bass_kernel_spmd(nc, in_maps, list(range(N_CORES)))
    return assemble_output(res.results)


if __name__ == "__main__":
    rng = np.random.default_rng(0)
    x = rng.standard_normal((BATCH, IN_F), dtype=np.float32)
    basis = (rng.standard_normal((BASIS, IN_F)) * 0.02).astype(np.float32)
    codes = rng.integers(0, 1 << 22, size=(OUT_F,), dtype=np.int32)
    y = kernel(x, codes, basis)

    idx = codes & 255
    r = ((codes >> 8) & 4095).astype(np.float32) / R_LEVELS
    sign = np.where(((codes >> 20) & 1) == 1, -1.0, 1.0).astype(np.float32)
    scale = sign * np.tanh(r)
    W = scale[:, None] * basis[idx]
    y_ref = x @ W.T
    err = np.linalg.norm(y - y_ref) / np.linalg.norm(y_ref)
    print("rel err:", err)



# revision 5
# speedup vs baseline: 1.2712x; 1.2712x over previous
"""Trainium2 Bass kernel for nn_BitfieldLinear (vq_codebook).

Reference computation:
    idx   = codes & 0xFF            (basis row, 256 entries)
    r_q   = (codes >> 8) & 0xFFF
    sign  = bit20 ? -1 : +1
    scale = sign * tanh(r_q / 4095)
    W     = scale[:, None] * basis[idx]        # [8192, 4096]
    y     = x @ W.T                            # [128, 8192]

Key factorization (never materialize the 128MB W):
    Z = x @ basis.T                            # [128, 256]  tiny matmul
    y[b, j] = scale[j] * Z[b, idx[j]]          # column gather + scale

The gather+scale is a matmul with a scaled one-hot matrix:
    G[k, j] = scale[j] * (idx[j] == k)         # [256, 1024] per core
    y_core  = Z @ G                            # [128, 1024]

Sharding: out_features column-parallel across 8 cores (1024 codes per
core); x and basis replicated (an 8-core AllReduce of Z measures ~68us
on this harness -- far slower than recomputing Z per core).

v2 layout (from baseline trace analysis; input stream runs at the
~358GB/s HBM/NC roofline, so the wins are scheduling, not bandwidth):
  - c128 (codes) issued FIRST on the sync HWDGE queue: the whole
    decode -> G chain depends only on it, and on the baseline it sat
    behind iota/ident on the slow SWDGE queue until ~12us.
  - iota / identity generated on-chip (gpsimd iota + DVE is_equal):
    two fewer DMAs, 192KB less HBM traffic.
  - tanh via degree-4 odd polynomial on DVE: no ACT table load, which
    also frees the scalar queue to start streaming x ~2us earlier.
  - G^T tiles, transposes, Z^T and the y matmuls all in bf16 (1 PE
    cycle/row vs 2 for f32 transposes, half-size copies); one-hot
    columns mean y has no accumulation-error amplification.
  - queue split: scalar=x (1MB), sync=c128+basis k0-15 (1MB),
    gpsimd=basis k16-31 (1MB); G transposes interleaved into the PE
    stream where DMA-paced gaps exist; output split across two HWDGE
    queues as soon as each 512-column PSUM bank closes.
QUANT="fp8" variant streams x/basis as fp8e3m4 (pre-scaled by 2/64,
compensated in the tanh coeffs): halves stream bytes at ~1.8% rel err.
"""

import sys

for _p in ("/opt/trn_rl_repo", "/opt/pypackages"):
    if _p not in sys.path:
        sys.path.insert(0, _p)

import numpy as np

import concourse.bacc as bacc
import concourse.mybir as mybir
import concourse.tile as tile
from concourse.alu_op_type import AluOpType
from concourse.bass_utils import run_bass_kernel_spmd

N_CORES = 8
BATCH = 128
IN_F = 4096
OUT_F = 8192
BASIS = 256
OPC = OUT_F // N_CORES      # 1024 output columns per core
NK = IN_F // 128            # 32 K-tiles
NT = OPC // 128             # 8 code-tiles per core
R_LEVELS = 4095.0

F32 = mybir.dt.float32
BF16 = mybir.dt.bfloat16
FP16 = mybir.dt.float16
FP8 = mybir.dt.float8e3
I32 = mybir.dt.int32

import os

QUANT = os.environ.get("BITF_QUANT", "fp16")   # "fp16" | "fp8" (fp8e3m4)
IN_DT = FP16 if QUANT == "fp16" else FP8
X_SCALE = 1.0 if QUANT == "fp16" else 2.0    # keep fp8e3m4 out of denormals
B_SCALE = 1.0 if QUANT == "fp16" else 64.0
_COMP = 1.0 / (X_SCALE * B_SCALE)            # folded into tanh coeffs

# tanh(r) ~= r*(c0 + c1 u + c2 u^2 + c3 u^3 + c4 u^4), u=r^2, r in [0,1]
# (max rel err 7e-6); coeffs carry the fp8 pre-scale compensation
TANH_C = [c * _COMP for c in (
    9.9999309235e-01, -3.3298076408e-01, 1.3036874854e-01,
    -4.4818694509e-02, 9.0370542166e-03)]

# K-tile ranges per DMA queue (first chunks small for an early PE start)
X_CHUNKS = [(0, 4), (4, 16), (16, 32)]        # scalar HWDGE queue
B_SYNC_CHUNKS = [(0, 2), (2, 8), (8, 16)]     # sync HWDGE queue
B_GPS_CHUNKS = [(16, 24), (24, 32)]           # gpsimd SWDGE queue

# after Z k-tile n, emit these G^T tile transposes on the PE
G_T_SCHED = {11: [0, 1], 13: [2, 3], 15: [4, 5], 17: [6, 7]}


def build_nc():
    nc = bacc.Bacc(
        "TRN2",
        target_bir_lowering=False,
        debug=False,
        num_devices=N_CORES,
    )

    xd = [
        nc.dram_tensor(f"xc{i}", [128, (e - s) * 128], IN_DT,
                       kind="ExternalInput")
        for i, (s, e) in enumerate(X_CHUNKS)
    ]
    bsd = [
        nc.dram_tensor(f"bsc{i}", [128, (e - s) * 256], IN_DT,
                       kind="ExternalInput")
        for i, (s, e) in enumerate(B_SYNC_CHUNKS)
    ]
    bgd = [
        nc.dram_tensor(f"bgc{i}", [128, (e - s) * 256], IN_DT,
                       kind="ExternalInput")
        for i, (s, e) in enumerate(B_GPS_CHUNKS)
    ]
    c128_d = nc.dram_tensor("c128", [128, NT], I32, kind="ExternalInput")
    out_d = nc.dram_tensor("out", [128, OPC], FP16, kind="ExternalOutput")

    with tile.TileContext(nc) as tc:
        with (
            tc.tile_pool(name="pool", bufs=1) as pool,
            tc.tile_pool(name="zps", bufs=1, space="PSUM") as zps,
            tc.tile_pool(name="tps", bufs=3, space="PSUM") as tps,
            tc.tile_pool(name="yps", bufs=1, space="PSUM") as yps,
        ):
            # ---- DMA issues: c128 first (gates the decode -> G chain)
            c128 = pool.tile([128, NT], I32)
            nc.sync.dma_start(out=c128[:], in_=c128_d[:])

            x_sb = pool.tile([128, IN_F], IN_DT)
            b_sb = pool.tile([128, 2 * IN_F], IN_DT)
            for i, (s, e) in enumerate(X_CHUNKS):
                nc.scalar.dma_start(out=x_sb[:, s * 128:e * 128], in_=xd[i][:])
            for i, (s, e) in enumerate(B_SYNC_CHUNKS):
                nc.sync.dma_start(out=b_sb[:, s * 256:e * 256], in_=bsd[i][:])

            # on-chip iota (gpsimd) before its basis DMAs
            iota_row_i = pool.tile([128, BASIS], I32)
            nc.gpsimd.iota(out=iota_row_i[:], pattern=[[1, BASIS]], base=0,
                           channel_multiplier=0)
            iota_part_i = pool.tile([128, 1], I32)
            nc.gpsimd.iota(out=iota_part_i[:], pattern=[[1, 1]], base=0,
                           channel_multiplier=1)
            for i, (s, e) in enumerate(B_GPS_CHUNKS):
                nc.gpsimd.dma_start(out=b_sb[:, s * 256:e * 256], in_=bgd[i][:])

            # ---- constants on DVE: iota_f [128,256], bf16 identity
            iota_f = pool.tile([128, BASIS], F32)
            nc.vector.tensor_scalar_mul(out=iota_f[:], in0=iota_row_i[:],
                                        scalar1=1.0)
            iota_part_f = pool.tile([128, 1], F32)
            nc.vector.tensor_scalar_mul(out=iota_part_f[:], in0=iota_part_i[:],
                                        scalar1=1.0)
            identb = pool.tile([128, 128], BF16)
            nc.vector.tensor_scalar(
                out=identb[:], in0=iota_f[:, 0:128],
                scalar1=iota_part_f[:, 0:1], scalar2=None,
                op0=AluOpType.is_equal,
            )

            # ---- decode codes -> idx_f (f32), scl (f32), both [128, NT]
            idx_f = pool.tile([128, NT], F32)
            scl = pool.tile([128, NT], F32)

            idx_i = pool.tile([128, NT], I32, name="idx_i")
            nc.vector.tensor_scalar(
                out=idx_i[:], in0=c128[:],
                scalar1=255, scalar2=None, op0=AluOpType.bitwise_and,
            )
            nc.vector.tensor_scalar_mul(out=idx_f[:], in0=idx_i[:], scalar1=1.0)
            rq_i = pool.tile([128, NT], I32, name="rq_i")
            nc.vector.tensor_scalar(
                out=rq_i[:], in0=c128[:],
                scalar1=8, scalar2=4095,
                op0=AluOpType.logical_shift_right,
                op1=AluOpType.bitwise_and,
            )
            r = pool.tile([128, NT], F32, name="r")
            nc.vector.tensor_scalar_mul(out=r[:], in0=rq_i[:],
                                        scalar1=1.0 / R_LEVELS)
            u = pool.tile([128, NT], F32, name="u")
            nc.vector.tensor_tensor(out=u[:], in0=r[:], in1=r[:],
                                    op=AluOpType.mult)
            # Horner: p = c4*u + c3; p = p*u + c2; ...; th = r*p
            p = pool.tile([128, NT], F32, name="p")
            nc.vector.tensor_scalar(
                out=p[:], in0=u[:], scalar1=TANH_C[4], scalar2=TANH_C[3],
                op0=AluOpType.mult, op1=AluOpType.add,
            )
            for ci in (2, 1, 0):
                nc.vector.tensor_tensor(out=p[:], in0=p[:], in1=u[:],
                                        op=AluOpType.mult)
                nc.vector.tensor_scalar(
                    out=p[:], in0=p[:], scalar1=TANH_C[ci], scalar2=None,
                    op0=AluOpType.add,
                )
            th = pool.tile([128, NT], F32, name="th")
            nc.vector.tensor_tensor(out=th[:], in0=p[:], in1=r[:],
                                    op=AluOpType.mult)
            sg_i = pool.tile([128, NT], I32, name="sg_i")
            nc.vector.tensor_scalar(
                out=sg_i[:], in0=c128[:],
                scalar1=20, scalar2=1,
                op0=AluOpType.logical_shift_right,
                op1=AluOpType.bitwise_and,
            )
            sgn = pool.tile([128, NT], F32, name="sgn")
            nc.vector.tensor_scalar(
                out=sgn[:], in0=sg_i[:],
                scalar1=-2.0, scalar2=1.0,
                op0=AluOpType.mult, op1=AluOpType.add,
            )
            nc.vector.tensor_tensor(out=scl[:], in0=th[:], in1=sgn[:],
                                    op=AluOpType.mult)

            # ---- G^T tiles (bf16): gt[p, k] = scl[t*128+p] * (idx==k)
            g_sb = [pool.tile([128, OPC], BF16, tag=f"g{h}", name=f"g_sb{h}")
                    for h in range(2)]
            gts = []
            for t in range(NT):
                gt = pool.tile([128, BASIS], BF16, tag=f"gt{t}", name=f"gt{t}")
                nc.vector.tensor_scalar(
                    out=gt[:], in0=iota_f[:],
                    scalar1=idx_f[:, t:t + 1], scalar2=scl[:, t:t + 1],
                    op0=AluOpType.is_equal, op1=AluOpType.mult,
                )
                gts.append(gt)

            def emit_g_transpose(t):
                for h in range(2):
                    tp = tps.tile([128, 128], BF16, tag="tp",
                                  name=f"tp{t}_{h}")
                    nc.tensor.transpose(
                        out=tp[:], in_=gts[t][:, h * 128:(h + 1) * 128],
                        identity=identb[:],
                    )
                    if h == 0:
                        nc.vector.tensor_copy(
                            out=g_sb[h][:, t * 128:(t + 1) * 128], in_=tp[:]
                        )
                    else:
                        nc.scalar.copy(
                            out=g_sb[h][:, t * 128:(t + 1) * 128], in_=tp[:]
                        )

            # ---- Z accumulation [128b, 256c] over 32 K-tiles, with G^T
            # transposes slotted into the DMA-paced PE gaps
            z_ps = zps.tile([128, BASIS], F32, tag="z")
            for k in range(NK):
                nc.tensor.matmul(
                    z_ps[:],
                    lhsT=x_sb[:, k * 128:(k + 1) * 128],
                    rhs=b_sb[:, k * 256:(k + 1) * 256],
                    start=(k == 0), stop=(k == NK - 1),
                )
                for t in G_T_SCHED.get(k, []):
                    emit_g_transpose(t)

            # ---- Z -> bf16, PE-transpose into Z^T halves
            z_sb = pool.tile([128, BASIS], BF16)
            nc.vector.tensor_copy(out=z_sb[:], in_=z_ps[:])
            zt = [pool.tile([128, 128], BF16, tag=f"zt{h}", name=f"zt{h}")
                  for h in range(2)]
            for h in range(2):
                ztp = tps.tile([128, 128], BF16, tag="tp", name=f"ztp{h}")
                nc.tensor.transpose(
                    out=ztp[:], in_=z_sb[:, h * 128:(h + 1) * 128],
                    identity=identb[:],
                )
                if h == 0:
                    nc.vector.tensor_copy(out=zt[h][:], in_=ztp[:])
                else:
                    nc.scalar.copy(out=zt[h][:], in_=ztp[:])

            # ---- y = Z^T.T @ G (bf16), two 512-column PSUM banks; store
            # each as soon as its copy lands, on separate HWDGE queues
            for nch in range(2):
                y_ps = yps.tile([128, 512], F32, tag=f"y{nch}",
                                name=f"y_ps{nch}")
                for h in range(2):
                    nc.tensor.matmul(
                        y_ps[:],
                        lhsT=zt[h][:],
                        rhs=g_sb[h][:, nch * 512:(nch + 1) * 512],
                        start=(h == 0), stop=(h == 1),
                    )
                y_sb = pool.tile([128, 512], FP16, tag=f"ysb{nch}",
                                 name=f"y_sb{nch}")
                if nch == 0:
                    nc.vector.tensor_copy(out=y_sb[:], in_=y_ps[:])
                    nc.scalar.dma_start(
                        out=out_d[:, 0:512], in_=y_sb[:]
                    )
                else:
                    nc.scalar.copy(out=y_sb[:], in_=y_ps[:])
                    nc.sync.dma_start(
                        out=out_d[:, 512:1024], in_=y_sb[:]
                    )

    nc.compile()
    return nc


_NC = None


def _get_nc():
    global _NC
    if _NC is None:
        _NC = build_nc()
    return _NC


def _np_in_dt():
    import ml_dtypes

    return np.float16 if QUANT == "fp16" else ml_dtypes.float8_e3m4


def make_in_maps(x, codes, basis):
    x = np.ascontiguousarray(x, dtype=np.float32)
    basis = np.ascontiguousarray(basis, dtype=np.float32)
    codes = np.ascontiguousarray(codes, dtype=np.int32)
    np_dt = _np_in_dt()

    # xt[p, k*128 + m] = x[m, k*128 + p]
    xt = np.ascontiguousarray(
        (x * X_SCALE).reshape(BATCH, NK, 128).transpose(2, 1, 0)
        .reshape(128, IN_F)
    ).astype(np_dt)
    # bt[p, k*256 + o] = basis[o, k*128 + p]
    bt = np.ascontiguousarray(
        (basis * B_SCALE).reshape(BASIS, NK, 128).transpose(2, 1, 0)
        .reshape(128, 2 * IN_F)
    ).astype(np_dt)

    shared = {}
    for i, (s, e) in enumerate(X_CHUNKS):
        shared[f"xc{i}"] = np.ascontiguousarray(xt[:, s * 128:e * 128])
    for i, (s, e) in enumerate(B_SYNC_CHUNKS):
        shared[f"bsc{i}"] = np.ascontiguousarray(bt[:, s * 256:e * 256])
    for i, (s, e) in enumerate(B_GPS_CHUNKS):
        shared[f"bgc{i}"] = np.ascontiguousarray(bt[:, s * 256:e * 256])

    in_maps = []
    for c in range(N_CORES):
        sh = codes[c * OPC:(c + 1) * OPC]
        # wrap-128 layout: c128[p, t] = codes[t*128 + p]
        c128 = np.ascontiguousarray(sh.reshape(NT, 128).T)
        in_maps.append({**shared, "c128": c128})
    return in_maps


def assemble_output(results):
    return np.concatenate(
        [results[c]["out"].astype(np.float32) for c in range(N_CORES)], axis=1
    )


def kernel(x, codes, basis):
    nc = _get_nc()
    in_maps = make_in_maps(x, codes, basis)
    res = run_bass_kernel_spmd(nc, in_maps, list(range(N_CORES)))
    return assemble_output(res.results)


if __name__ == "__main__":
    rng = np.random.default_rng(0)
    x = rng.standard_normal((BATCH, IN_F), dtype=np.float32)
    basis = (rng.standard_normal((BASIS, IN_F)) * 0.02).astype(np.float32)
    codes = rng.integers(0, 1 << 22, size=(OUT_F,), dtype=np.int32)
    y = kernel(x, codes, basis)

    idx = codes & 255
    r = ((codes >> 8) & 4095).astype(np.float32) / R_LEVELS
    sign = np.where(((codes >> 20) & 1) == 1, -1.0, 1.0).astype(np.float32)
    scale = sign * np.tanh(r)
    W = scale[:, None] * basis[idx]
    y_ref = x @ W.T
    err = np.linalg.norm(y - y_ref) / np.linalg.norm(y_ref)
    print("rel err:", err)
